# revision 13
# baseline (speedup 1.0000x reference)
"""DarkChannel kernel for Trainium2: channel-min + 15x15 separable min-pool.

Full input img [16, 3, 1024, 1024] f32 -> output [16, 1, 1024, 1024] f32.
Batch-sharded across 8 NeuronCores (2 images per core), bf16 datapath
(tolerance 2e-2 >> bf16 rounding ~2e-3).

Per-core pipeline, per row-block (114 output rows, 9 blocks):
  - SWDGE cast loads f32->bf16: 128 input rows (7-row halo; out-of-range
    rows become +inf via memset, matching the reference +inf border)
    x 3 channels x 2 images
  - channel min: 2 DVE tensor_tensor min (bf16, 2x mode)
  - horizontal window-15 min: 4-op doubling tree (shifts 1,2,4,7) on DVE;
    odd free offsets measured to keep 2x mode on this HW
  - orientation flip: one chunked DMA transpose [128,2048]->[128,16,128]
    (xbar, HWDGE) so rows land in the free dim
  - vertical window-15 min: 4-op doubling tree on DVE in transposed
    layout (full-width ops; garbage beyond row 114 is discarded)
  - transpose back, SWDGE cast store bf16->f32
"""
import sys
sys.path.insert(0, '/opt/trn_rl_repo')

import numpy as np

import concourse.bacc as bacc_mod
import concourse.mybir as mybir
from concourse.tile import TileContext
from concourse import bass_utils

F32 = mybir.dt.float32
BF16 = mybir.dt.bfloat16
MIN = mybir.AluOpType.min
INF = float('inf')

H = 1024
W = 1024
C = 3
NIMG = 2              # images per core
N_CORES = 8
RBLK = 114            # output rows per block
NBLK = 9
LP = 8                # left pad (>=7), image pitch stays even for bf16
IP = 1040             # per-image pitch in padded x tile
K = 16                # 128-col transpose chunks (2 images x 8)
VP = 160              # transposed-tile inner pitch (xbar writes 32-col tiles,
                      # so the per-chunk pitch must be a multiple of 32)

_cache = {}


def _build():
    nc = bacc_mod.Bacc("TRN2", target_bir_lowering=False, debug=False,
                       num_devices=N_CORES)
    img = nc.dram_tensor("img", [NIMG, C, H, W], F32, kind="ExternalInput")
    infc = nc.dram_tensor("infc", [1, NIMG * IP], BF16, kind="ExternalInput")
    out = nc.dram_tensor("out", [NIMG, 1, H, W], F32, kind="ExternalOutput")

    with TileContext(nc) as tc:
        with tc.tile_pool(name="chin", bufs=4) as chpool, \
             tc.tile_pool(name="work", bufs=4) as wpool, \
             tc.tile_pool(name="vwork", bufs=4) as vpool:

            def emit_loads(b):
                lo = b * RBLK - 7
                ct = chpool.tile([128, C, NIMG, W], BF16, tag="ch")
                src_lo = max(lo, 0)
                src_hi = min(lo + 128, H)
                d0 = src_lo - lo
                for i in range(NIMG):
                    # one cast DMA per image: channels ride the middle dim
                    nc.gpsimd.dma_start(
                        out=ct[d0:d0 + (src_hi - src_lo), :, i, :],
                        in_=img[i, :, src_lo:src_hi, :]
                            .rearrange("c r w -> r c w"))
                return ct

            pending = [emit_loads(0), emit_loads(1), emit_loads(2)]

            def front(b, ct):
                # chmin + h-tree + forward transpose for block b
                lo = b * RBLK - 7
                x = wpool.tile([128, NIMG, IP], BF16, tag="x")
                nc.gpsimd.memset(x[:, :, 0:LP], INF)
                nc.gpsimd.memset(x[:, :, LP + W:IP], INF)
                mid = x[:, :, LP:LP + W]
                nc.vector.tensor_tensor(mid, ct[:, 0, :, :], ct[:, 1, :, :],
                                        MIN)
                nc.vector.tensor_tensor(mid, mid, ct[:, 2, :, :], MIN)
                if lo < 0:  # top halo rows -> +inf border
                    nc.gpsimd.memset(x[0:-lo, :, :], INF)
                if lo + 128 > H:  # bottom halo rows (memset can't start at
                    # partition 119; broadcast-DMA +inf instead)
                    nh = 128 - (H - lo)
                    nc.sync.dma_start(
                        out=x[H - lo:128, :, :],
                        in_=infc[0:1, :].to_broadcast((nh, NIMG * IP)))
                # horizontal window-15 min, causal tree (t1 reuses x, t2
                # reuses y; hm[j] = min x cols [j-7 .. j+7])
                y = wpool.tile([128, NIMG, IP], BF16, tag="y")
                hm = wpool.tile([128, NIMG, W], BF16, tag="hm")
                nc.vector.tensor_tensor(y[:, :, 0:1038], x[:, :, 1:1039],
                                        x[:, :, 2:1040], MIN)
                nc.vector.tensor_tensor(x[:, :, 0:1036], y[:, :, 0:1036],
                                        y[:, :, 2:1038], MIN)
                nc.vector.tensor_tensor(y[:, :, 0:1032], x[:, :, 0:1032],
                                        x[:, :, 4:1036], MIN)
                nc.vector.tensor_tensor(hm[:, :, 0:W], y[:, :, 0:1024],
                                        y[:, :, 7:1031], MIN)
                vt = vpool.tile([128, K, VP], BF16, tag="vt")
                nc.sync.dma_start(out=vt[:, :, 0:128],
                                  in_=hm[:].rearrange("p i w -> p (i w)"),
                                  transpose=True)
                return vt

            vts = [front(0, pending.pop(0))]
            for b in range(NBLK):
                r0 = b * RBLK
                rout = min(RBLK, H - r0)
                if b + 3 < NBLK:
                    pending.append(emit_loads(b + 3))
                if b + 1 < NBLK:
                    vts.append(front(b + 1, pending.pop(0)))
                vt = vts.pop(0)

                # vertical window-15 min: doubling tree along free dim.
                # Full-width ops; outputs at r >= 114 read stale slack
                # columns and are discarded after the transpose back.
                va = vpool.tile([128, K, VP], BF16, tag="va")
                vo = vpool.tile([128, K, 128], BF16, tag="vo")
                nc.vector.tensor_tensor(va[:, :, 0:128], vt[:, :, 0:128],
                                        vt[:, :, 1:129], MIN)
                nc.vector.tensor_tensor(vt[:, :, 0:128], va[:, :, 0:128],
                                        va[:, :, 2:130], MIN)
                nc.vector.tensor_tensor(va[:, :, 0:128], vt[:, :, 0:128],
                                        vt[:, :, 4:132], MIN)
                nc.vector.tensor_tensor(vo[:, :, 0:128], va[:, :, 0:128],
                                        va[:, :, 7:135], MIN)

                # back to natural layout, then cast store
                nb = vpool.tile([128, NIMG, W], BF16, tag="nb")
                nc.sync.dma_start(
                    out=nb[:].rearrange("p i w -> p (i w)")
                        .rearrange("p (k q) -> p k q", k=K),
                    in_=vo[:].rearrange("p k q -> p (k q)"),
                    transpose=True)
                for i in range(NIMG):
                    nc.gpsimd.dma_start(out=out[i, 0, r0:r0 + rout, :],
                                        in_=nb[0:rout, i, :])

    nc.compile()
    return nc


def kernel(img: np.ndarray) -> np.ndarray:
    assert img.shape == (16, 3, 1024, 1024) and img.dtype == np.float32
    if "nc" not in _cache:
        _cache["nc"] = _build()
    nc = _cache["nc"]
    infc = np.full((1, NIMG * IP), np.inf, dtype=mybir.dt.np(BF16))
    in_maps = [{"img": np.ascontiguousarray(img[2 * k:2 * k + 2]),
                "infc": infc}
               for k in range(N_CORES)]
    res = bass_utils.run_bass_kernel_spmd(
        nc, in_maps, core_ids=list(range(N_CORES)))
    return np.concatenate([r["out"] for r in res.results], axis=0)
